# revision 19
# baseline (speedup 1.0000x reference)
"""Trainium2 Bass kernel for cross-attention (efficient/linear attention variant).

Computation per batch b (fully batch-independent -> data parallel over 8 cores):
    q  = Wq @ x[b]                         # (128, N)
    kv = Wkv @ context[b].T                # (256, NCTX)
    k, v = kv[:128], kv[128:]
    q = softmax_d(q) * d**-0.5             # softmax over feature dim within head
    k = softmax_n(k)                       # softmax over sequence dim
    ctx[h] = k_h @ v_h.T                   # (32, 32) per head
    out[h] = ctx[h].T @ q_h                # (32, N)
    y = Wo @ out + bo

Strategy (v4):
  - One batch per NeuronCore (8 cores), no collectives.
  - Host pre-transposes/tiles everything so all device DMAs are fully
    contiguous per partition; streams in bf16.
  - Merged phase A+B1 loop over 32 tiles: kvT = ctxT_chunk.T @ WkvT (n on
    partitions), exp(k) on ScalarE (one batched ACTIVATE per tile), one
    accumulating matmul per 128-chunk computes C[(d),(e)] = sum_n exp(k) v
    AND Z[d] = sum_n exp(k) via a ones column on v.  Concurrently (B1) the
    q-projection, exp(q), S = head-sums of exp(q) (computed PRE-BROADCAST
    to all 128 partitions via a block-mask matmul), and 1/S via
    reciprocal_approx_fast on DVE run on the same tiles using spare
    Scalar/DVE/PE capacity under the kv-matmul stream.  (A (4,n)-shaped
    reciprocal would serialize ~128us on 4 partitions; the broadcast-first
    form keeps all 128 DVE lanes busy.)
  - Barrier: BD = blockdiag(C/Z); M = BD^T @ (Wo^T * scale) folded once on
    device (PE transpose + 1 matmul), so phase B2 has no ctx matmul.
  - Phase B2 (2-stage pipeline): eqn = exp(q) * (1/S) on DVE,
    y = M^T @ eqn, one fused PSUM->SBUF evacuation on ScalarE (bias folded
    in; plain copy when bo == 0), DMA out.
    No Ln/Reciprocal activations anywhere -> zero ACT table swaps.
"""

import sys
from contextlib import ExitStack

import numpy as np

if "/opt/trn_rl_repo" not in sys.path:
    sys.path.insert(0, "/opt/trn_rl_repo")

import ml_dtypes

import concourse.bass as bass
from concourse import bacc
import concourse.mybir as mybir
import concourse.tile as tile
from concourse.bass_utils import run_bass_kernel_spmd

HEADS = 4
DIM_HEAD = 32
SCALE = DIM_HEAD**-0.5
B = 8
DIM = 256
N = 16384
NCTX = 16384
CDIM = 512
HID = HEADS * DIM_HEAD  # 128

BF16 = mybir.dt.bfloat16
F32 = mybir.dt.float32
EXP = mybir.ActivationFunctionType.Exp

TILE_N = 512
NT = N // TILE_N  # 32 x-tiles (== context tiles)
NCHUNK = NCTX // 128  # 128 chunks in the C/Z accumulation


def build_graph(bias_zero: bool) -> bass.Bass:
    nc = bacc.Bacc()

    ctxt = nc.dram_tensor("ctxt", [NT, 128, 4 * TILE_N], BF16, kind="ExternalInput")
    xst = nc.dram_tensor("xst", [NT, 128, 2 * TILE_N], BF16, kind="ExternalInput")
    wqt = nc.dram_tensor("wqt", [128, 2, HID], BF16, kind="ExternalInput")
    wkvt = nc.dram_tensor("wkvt", [128, 4, 2 * HID], BF16, kind="ExternalInput")
    wot = nc.dram_tensor("wot", [HID, DIM], BF16, kind="ExternalInput")
    bob = nc.dram_tensor("bob", [128, 2], F32, kind="ExternalInput")
    bmask = nc.dram_tensor("bmask", [HID, HID], BF16, kind="ExternalInput")
    ident = nc.dram_tensor("ident", [HID, HID], BF16, kind="ExternalInput")
    y = nc.dram_tensor("y", [NT, 128, 2 * TILE_N], BF16, kind="ExternalOutput")

    with tile.TileContext(nc) as tc, ExitStack() as ctx:
        cpool = ctx.enter_context(tc.tile_pool(name="consts", bufs=1))

        # wkvt first: needed by the very first matmul.
        wkvt_sb = cpool.tile([128, 4, 2 * HID], BF16)
        nc.sync.dma_start(wkvt_sb, wkvt[:, :, :])

        # persistent intermediates
        eq_all = cpool.tile([128, NT, TILE_N], BF16)  # exp(q), 32KB/part
        eqn_all = cpool.tile([128, NT, TILE_N], BF16)  # exp(q)/S, 32KB/part
        m_sb = cpool.tile([HID, DIM], BF16)  # folded BD^T @ WoT
        bd_sb = cpool.tile([HID, HID], BF16)
        bdt_sb = cpool.tile([HID, HID], BF16)

        # manual vt rotation: ones column at [:, :, HID] preset once
        vt_bufs = [cpool.tile([128, 4, 136], BF16, name=f"vtb{i}") for i in range(3)]
        for vtb in vt_bufs:
            nc.gpsimd.memset(vtb[:, :, HID : HID + 1], 1.0)

        czctx = ExitStack()
        czpool = czctx.enter_context(tc.tile_pool(name="czp", bufs=1, space="PSUM"))
        cz_ps = czpool.tile([128, HID + 1], F32)

        with (
            tc.tile_pool(name="actx", bufs=4) as apool,
            tc.tile_pool(name="axs", bufs=6) as xpool,
            tc.tile_pool(name="akv", bufs=3) as kpool,
            tc.tile_pool(name="aps", bufs=2, space="PSUM") as apsum,
        ):
            # context tile 0 (split in two for earlier compute start) + x0
            ct_tiles: dict = {}
            xs_tiles: dict = {}
            ct0 = apool.tile([128, 4 * TILE_N], BF16, tag="ct")
            ct0_r = ct0.rearrange("p (c n) -> p c n", c=4)
            src0 = ctxt[0].rearrange("p (c n) -> p c n", c=4)
            # first context tile issued on the scalar HWDGE ring, in parallel
            # with wkvt on the sync ring, so the first matmul starts earlier
            nc.scalar.dma_start(ct0_r[:, :, 0:256], src0[:, :, 0:256])
            nc.scalar.dma_start(ct0_r[:, :, 256:TILE_N], src0[:, :, 256:TILE_N])
            xs0 = xpool.tile([128, 2 * TILE_N], BF16, tag="xt")
            nc.sync.dma_start(xs0, xst[0])
            ct_tiles[0] = ct0
            xs_tiles[0] = xs0

            wqt_sb = cpool.tile([128, 2, HID], BF16)
            nc.scalar.dma_start(wqt_sb, wqt[:, :, :])
            wot_sb = cpool.tile([HID, DIM], BF16)
            nc.sync.dma_start(wot_sb, wot[:, :])
            bo_sb = cpool.tile([128, 2], F32)
            nc.sync.dma_start(bo_sb, bob[:, :])
            bmask_sb = cpool.tile([HID, HID], BF16)
            nc.scalar.dma_start(bmask_sb, bmask[:, :])
            ident_sb = cpool.tile([HID, HID], BF16)
            nc.sync.dma_start(ident_sb, ident[:, :])

            # HAM pre-warm: junk matmuls on uninitialized SBUF fill the
            # otherwise-idle PE window while the first DMAs land, so the
            # clock gate opens (K=8/8) before real work starts.
            warm_ps = apsum.tile([128, 4, 2 * HID], F32, tag="kvt")
            for w in range(14):
                nc.tensor.matmul(
                    warm_ps[:, 0, :],
                    eq_all[:, 20, 0:HID],
                    eq_all[:, 21, 0 : 2 * HID],
                    start=True,
                    stop=True,
                )

            # ------- merged Phase A + B1 (C/S work staggered one tile) ------
            kts: dict = {}
            rss: dict = {}
            for t in range(NT + 2):
                if t < NT:
                    if t not in ct_tiles:
                        ct = apool.tile([128, 4 * TILE_N], BF16, tag="ct")
                        nc.sync.dma_start(ct, ctxt[t])
                        xs = xpool.tile([128, 2 * TILE_N], BF16, tag="xt")
                        nc.sync.dma_start(xs, xst[t])
                        ct_tiles[t] = ct
                        xs_tiles[t] = xs
                    ct = ct_tiles.pop(t)
                    xs = xs_tiles.pop(t)
                    # kv projection: 4 chunks of 128 n, contraction over 4 cc
                    kvt_ps = apsum.tile([128, 4, 2 * HID], F32, tag="kvt")
                    for j in range(4):
                        for cc in range(4):
                            nc.tensor.matmul(
                                kvt_ps[:, j, :],
                                ct[:, cc * TILE_N + j * 128 : cc * TILE_N + (j + 1) * 128],
                                wkvt_sb[:, cc, :],
                                start=(cc == 0),
                                stop=(cc == 3),
                            )
                    kt = kpool.tile([128, 4, HID], BF16, tag="kt")
                    nc.scalar.activation(kt, kvt_ps[:, :, 0:HID], EXP)
                    vt = vt_bufs[t % 3]
                    nc.vector.tensor_copy(vt[:, :, 0:HID], kvt_ps[:, :, HID : 2 * HID])
                    kts[t] = kt
                    # B1: q projection + exp(q)
                    q_ps = apsum.tile([128, TILE_N], F32, tag="q")
                    for cc in range(2):
                        nc.tensor.matmul(
                            q_ps,
                            wqt_sb[:, cc, :],
                            xs[:, cc * TILE_N : (cc + 1) * TILE_N],
                            start=(cc == 0),
                            stop=(cc == 1),
                        )
                    nc.scalar.activation(eq_all[:, t, :], q_ps, EXP)
                if 1 <= t <= NT:
                    u = t - 1
                    kt = kts.pop(u)
                    vt = vt_bufs[u % 3]
                    for j in range(4):
                        ci = u * 4 + j
                        nc.tensor.matmul(
                            cz_ps,
                            kt[:, j, :],
                            vt[:, j, 0 : HID + 1],
                            start=(ci == 0),
                            stop=(ci == NCHUNK - 1),
                        )
                    # B1: S broadcast to 128 rows via block mask, then 1/S
                    sb_ps = apsum.tile([128, TILE_N], F32, tag="sb", bufs=1)
                    nc.tensor.matmul(
                        sb_ps, bmask_sb, eq_all[:, u, :], start=True, stop=True
                    )
                    rs = kpool.tile([128, TILE_N], F32, tag="rs")
                    nc.vector.reciprocal_approx_fast(rs, sb_ps)
                    rss[u] = rs
                if t >= 2:
                    u2 = t - 2
                    nc.vector.tensor_mul(
                        eqn_all[:, u2, :], eq_all[:, u2, :], rss.pop(u2)
                    )

        # ------- barrier: M = blockdiag(C/Z)^T @ WoT ------------------------
        with tc.tile_pool(name="barp", bufs=1, space="PSUM") as barpsum:
            # keep the PE clock gate open across the barrier's DVE chain
            bwarm_ps = barpsum.tile([128, TILE_N], F32)
            for w in range(8):
                nc.tensor.matmul(
                    bwarm_ps,
                    eq_all[:, 20, 0:HID],
                    eq_all[:, 21, :],
                    start=True,
                    stop=True,
                )
            rz = cpool.tile([128, 1], F32)
            nc.vector.reciprocal(rz, cz_ps[:, HID : HID + 1])
            bdf = cpool.tile([128, HID], F32)
            nc.vector.tensor_scalar_mul(bdf, cz_ps[:, 0:HID], rz)
            nc.vector.tensor_mul(bd_sb, bdf, bmask_sb)
            bdt_ps = barpsum.tile([128, HID], BF16)
            nc.tensor.transpose(bdt_ps, bd_sb, ident_sb)
            nc.vector.tensor_copy(bdt_sb, bdt_ps)
            m_ps = barpsum.tile([128, DIM], F32)
            nc.tensor.matmul(m_ps, bdt_sb, wot_sb, start=True, stop=True)
            nc.vector.tensor_copy(m_sb, m_ps)
        czctx.close()

        # ------- Phase B2: y = M^T @ eqn, evacuation split ScalarE/DVE ------
        with (
            tc.tile_pool(name="bsb", bufs=3) as bpool,
            tc.tile_pool(name="bps", bufs=2, space="PSUM") as bpsum,
        ):
            ywarm_ps = bpsum.tile([128, TILE_N], F32, tag="warm", bufs=1)
            for t in range(NT):
                y_ps = bpsum.tile([128, 2, TILE_N], F32, tag="y")
                for oc in range(2):
                    nc.tensor.matmul(
                        y_ps[:, oc, :],
                        m_sb[:, oc * HID : (oc + 1) * HID],
                        eqn_all[:, t, :],
                        start=True,
                        stop=True,
                    )
                # duty-cycle filler: keeps HAM at K=8/8 through the
                # evacuation-bound stretch so real matmuls run at 2.4 GHz
                nc.tensor.matmul(
                    ywarm_ps,
                    eq_all[:, 20, 0:HID],
                    eq_all[:, t, :],
                    start=True,
                    stop=True,
                )
                yt = bpool.tile([128, 2 * TILE_N], BF16, tag="yt")
                if bias_zero:
                    nc.scalar.copy(yt[:, 0:TILE_N], y_ps[:, 0, :])
                    nc.vector.tensor_copy(yt[:, TILE_N:], y_ps[:, 1, :])
                else:
                    nc.scalar.add(yt[:, 0:TILE_N], y_ps[:, 0, :], bo_sb[:, 0:1])
                    nc.vector.tensor_scalar_add(
                        yt[:, TILE_N:], y_ps[:, 1, :], bo_sb[:, 1:2]
                    )
                nc.sync.dma_start(y[t], yt)

    nc.compile()
    return nc


_GRAPH_CACHE: dict = {}


def _prep_inputs(x, context, Wq, Wkv, Wo, bo):
    bf16 = ml_dtypes.bfloat16
    x = np.asarray(x, dtype=np.float32)
    context = np.asarray(context, dtype=np.float32)
    Wq = np.asarray(Wq, dtype=np.float32)
    Wkv = np.asarray(Wkv, dtype=np.float32)
    Wo = np.asarray(Wo, dtype=np.float32)
    bo = np.asarray(bo, dtype=np.float32)

    # [128, 2, HID]: wqt[p, cc, m] = Wq[m, cc*128+p]
    wqt = np.ascontiguousarray(Wq.T.reshape(2, 128, HID).transpose(1, 0, 2)).astype(bf16)
    # [128, 4, 256]: wkvt[p, cc, o] = Wkv[o, cc*128+p]
    wkvt = np.ascontiguousarray(Wkv.T.reshape(4, 128, 2 * HID).transpose(1, 0, 2)).astype(bf16)
    # [HID, DIM]: wot[e, o] = Wo[o, e] * SCALE
    wot = np.ascontiguousarray((Wo * SCALE).T).astype(bf16)
    bob = np.ascontiguousarray(bo.reshape(2, 128).T).astype(np.float32)

    bmask = (
        (np.arange(HID)[:, None] // DIM_HEAD) == (np.arange(HID)[None, :] // DIM_HEAD)
    ).astype(bf16)
    ident = np.eye(HID, dtype=bf16)

    in_maps = []
    for b in range(B):
        # ctxt[t, p, cc*512+j] = context[b, t*512+j, cc*128+p]
        ctx_t = np.ascontiguousarray(
            context[b].reshape(NT, TILE_N, 4, 128).transpose(0, 3, 2, 1).reshape(NT, 128, 4 * TILE_N)
        ).astype(bf16)
        # xst[t, p, cc*512+j] = x[b, cc*128+p, t*512+j]
        xs_t = np.ascontiguousarray(
            x[b].reshape(2, 128, NT, TILE_N).transpose(2, 1, 0, 3).reshape(NT, 128, 2 * TILE_N)
        ).astype(bf16)
        in_maps.append(
            {
                "ctxt": ctx_t,
                "xst": xs_t,
                "wqt": wqt,
                "wkvt": wkvt,
                "wot": wot,
                "bob": bob,
                "bmask": bmask,
                "ident": ident,
            }
        )
    return in_maps


def run(inputs: dict, trace: bool = False):
    bias_zero = bool(np.all(np.asarray(inputs["bo"]) == 0))
    key = ("nc", bias_zero)
    if key not in _GRAPH_CACHE:
        _GRAPH_CACHE[key] = build_graph(bias_zero)
    nc = _GRAPH_CACHE[key]
    in_maps = _prep_inputs(**inputs)
    res = run_bass_kernel_spmd(nc, in_maps, core_ids=list(range(B)), trace=trace)
    out = np.stack(
        [
            np.asarray(res.results[b]["y"], dtype=np.float32)
            .reshape(NT, 128, 2, TILE_N)
            .transpose(2, 1, 0, 3)
            .reshape(DIM, N)
            for b in range(B)
        ]
    )
    return out, res


def kernel(**inputs) -> np.ndarray:
    out, _ = run(inputs, trace=False)
    return out


# revision 20
# speedup vs baseline: 1.1819x; 1.1819x over previous
"""Trainium2 Bass kernel for cross-attention (efficient/linear attention variant).

Computation per batch b (fully batch-independent -> data parallel over 8 cores):
    q  = Wq @ x[b]                         # (128, N)
    kv = Wkv @ context[b].T                # (256, NCTX)
    k, v = kv[:128], kv[128:]
    q = softmax_d(q) * d**-0.5             # softmax over feature dim within head
    k = softmax_n(k)                       # softmax over sequence dim
    ctx[h] = k_h @ v_h.T                   # (32, 32) per head
    out[h] = ctx[h].T @ q_h                # (32, N)
    y = Wo @ out + bo

Strategy (v4):
  - One batch per NeuronCore (8 cores), no collectives.
  - Host pre-transposes/tiles everything so all device DMAs are fully
    contiguous per partition; streams in bf16.
  - Merged phase A+B1 loop over 32 tiles: kvT = ctxT_chunk.T @ WkvT (n on
    partitions), exp(k) on ScalarE (one batched ACTIVATE per tile), one
    accumulating matmul per 128-chunk computes C[(d),(e)] = sum_n exp(k) v
    AND Z[d] = sum_n exp(k) via a ones column on v.  Concurrently (B1) the
    q-projection, exp(q), S = head-sums of exp(q) (computed PRE-BROADCAST
    to all 128 partitions via a block-mask matmul), and 1/S via
    reciprocal_approx_fast on DVE run on the same tiles using spare
    Scalar/DVE/PE capacity under the kv-matmul stream.  (A (4,n)-shaped
    reciprocal would serialize ~128us on 4 partitions; the broadcast-first
    form keeps all 128 DVE lanes busy.)
  - Barrier: BD = blockdiag(C/Z); M = BD^T @ (Wo^T * scale) folded once on
    device (PE transpose + 1 matmul), so phase B2 has no ctx matmul.
  - Phase B2 (2-stage pipeline): eqn = exp(q) * (1/S) on DVE,
    y = M^T @ eqn, one fused PSUM->SBUF evacuation on ScalarE (bias folded
    in; plain copy when bo == 0), DMA out.
    No Ln/Reciprocal activations anywhere -> zero ACT table swaps.
"""

import sys
from contextlib import ExitStack

import numpy as np

if "/opt/trn_rl_repo" not in sys.path:
    sys.path.insert(0, "/opt/trn_rl_repo")

import ml_dtypes

import concourse.bass as bass
from concourse import bacc
import concourse.mybir as mybir
import concourse.tile as tile
from concourse.bass_utils import run_bass_kernel_spmd

HEADS = 4
DIM_HEAD = 32
SCALE = DIM_HEAD**-0.5
B = 8
DIM = 256
N = 16384
NCTX = 16384
CDIM = 512
HID = HEADS * DIM_HEAD  # 128

BF16 = mybir.dt.bfloat16
F32 = mybir.dt.float32
FP8 = mybir.dt.float8e3
EXP = mybir.ActivationFunctionType.Exp

TILE_N = 512
NT = N // TILE_N  # 32 x-tiles (== context tiles)
NCHUNK = NCTX // 128  # 128 chunks in the C/Z accumulation


def build_graph(bias_zero: bool) -> bass.Bass:
    nc = bacc.Bacc()

    ctxt = nc.dram_tensor("ctxt", [NT, 128, 4 * TILE_N], FP8, kind="ExternalInput")
    xst = nc.dram_tensor("xst", [NT, 128, 2 * TILE_N], BF16, kind="ExternalInput")
    wqt = nc.dram_tensor("wqt", [128, 2, HID], BF16, kind="ExternalInput")
    wkvt = nc.dram_tensor("wkvt", [128, 4, 2 * HID], BF16, kind="ExternalInput")
    wot = nc.dram_tensor("wot", [HID, DIM], BF16, kind="ExternalInput")
    bob = nc.dram_tensor("bob", [128, 2], F32, kind="ExternalInput")
    bmask = nc.dram_tensor("bmask", [HID, HID], BF16, kind="ExternalInput")
    ident = nc.dram_tensor("ident", [HID, HID], BF16, kind="ExternalInput")
    y = nc.dram_tensor("y", [NT, 128, 2 * TILE_N], BF16, kind="ExternalOutput")

    with tile.TileContext(nc) as tc, ExitStack() as ctx:
        cpool = ctx.enter_context(tc.tile_pool(name="consts", bufs=1))

        # wkvt first: needed by the very first matmul.
        wkvt_sb = cpool.tile([128, 4, 2 * HID], BF16)
        nc.sync.dma_start(wkvt_sb, wkvt[:, :, :])

        # persistent intermediates
        eq_all = cpool.tile([128, NT, TILE_N], BF16)  # exp(q), 32KB/part
        eqn_all = cpool.tile([128, NT, TILE_N], BF16)  # exp(q)/S, 32KB/part
        m_sb = cpool.tile([HID, DIM], BF16)  # folded BD^T @ WoT
        bd_sb = cpool.tile([HID, HID], BF16)
        bdt_sb = cpool.tile([HID, HID], BF16)

        # manual vt rotation: ones column at [:, :, HID] preset once
        vt_bufs = [cpool.tile([128, 4, 136], BF16, name=f"vtb{i}") for i in range(3)]
        for vtb in vt_bufs:
            nc.gpsimd.memset(vtb[:, :, HID : HID + 1], 1.0)

        czctx = ExitStack()
        czpool = czctx.enter_context(tc.tile_pool(name="czp", bufs=1, space="PSUM"))
        cz_ps = czpool.tile([128, HID + 1], F32)

        with (
            tc.tile_pool(name="actx", bufs=4) as apool,
            tc.tile_pool(name="axs", bufs=6) as xpool,
            tc.tile_pool(name="akv", bufs=3) as kpool,
            tc.tile_pool(name="aps", bufs=2, space="PSUM") as apsum,
        ):
            # context tile 0 (split in two for earlier compute start) + x0
            ct_tiles: dict = {}
            xs_tiles: dict = {}
            ct0 = apool.tile([128, 4 * TILE_N], FP8, tag="ct")
            ct0_r = ct0.rearrange("p (c n) -> p c n", c=4)
            src0 = ctxt[0].rearrange("p (c n) -> p c n", c=4)
            # first context tile issued on the scalar HWDGE ring, in parallel
            # with wkvt on the sync ring, so the first matmul starts earlier
            nc.scalar.dma_start(ct0_r[:, :, 0:256], src0[:, :, 0:256])
            nc.scalar.dma_start(ct0_r[:, :, 256:TILE_N], src0[:, :, 256:TILE_N])
            xs0 = xpool.tile([128, 2 * TILE_N], BF16, tag="xt")
            nc.sync.dma_start(xs0, xst[0])
            ct_tiles[0] = ct0
            xs_tiles[0] = xs0

            wqt_sb = cpool.tile([128, 2, HID], BF16)
            nc.scalar.dma_start(wqt_sb, wqt[:, :, :])
            wot_sb = cpool.tile([HID, DIM], BF16)
            nc.sync.dma_start(wot_sb, wot[:, :])
            bo_sb = cpool.tile([128, 2], F32)
            nc.sync.dma_start(bo_sb, bob[:, :])
            bmask_sb = cpool.tile([HID, HID], BF16)
            nc.scalar.dma_start(bmask_sb, bmask[:, :])
            ident_sb = cpool.tile([HID, HID], BF16)
            nc.sync.dma_start(ident_sb, ident[:, :])

            # ------- merged Phase A + B1 (C/S work staggered one tile) ------
            kts: dict = {}
            rss: dict = {}
            for t in range(NT + 2):
                if t < NT:
                    if t not in ct_tiles:
                        ct = apool.tile([128, 4 * TILE_N], FP8, tag="ct")
                        nc.sync.dma_start(ct, ctxt[t])
                        xs = xpool.tile([128, 2 * TILE_N], BF16, tag="xt")
                        nc.sync.dma_start(xs, xst[t])
                        ct_tiles[t] = ct
                        xs_tiles[t] = xs
                    ct = ct_tiles.pop(t)
                    xs = xs_tiles.pop(t)
                    # kv projection: 4 chunks of 128 n, contraction over 4 cc
                    kvt_ps = apsum.tile([128, 4, 2 * HID], F32, tag="kvt")
                    for j in range(4):
                        for cc in range(4):
                            nc.tensor.matmul(
                                kvt_ps[:, j, :],
                                ct[:, cc * TILE_N + j * 128 : cc * TILE_N + (j + 1) * 128],
                                wkvt_sb[:, cc, :],
                                start=(cc == 0),
                                stop=(cc == 3),
                            )
                    kt = kpool.tile([128, 4, HID], BF16, tag="kt")
                    nc.scalar.activation(kt, kvt_ps[:, :, 0:HID], EXP)
                    vt = vt_bufs[t % 3]
                    nc.vector.tensor_copy(vt[:, :, 0:HID], kvt_ps[:, :, HID : 2 * HID])
                    kts[t] = kt
                    # B1: q projection + exp(q)
                    q_ps = apsum.tile([128, TILE_N], F32, tag="q")
                    for cc in range(2):
                        nc.tensor.matmul(
                            q_ps,
                            wqt_sb[:, cc, :],
                            xs[:, cc * TILE_N : (cc + 1) * TILE_N],
                            start=(cc == 0),
                            stop=(cc == 1),
                        )
                    nc.scalar.activation(eq_all[:, t, :], q_ps, EXP)
                if 1 <= t <= NT:
                    u = t - 1
                    kt = kts.pop(u)
                    vt = vt_bufs[u % 3]
                    for j in range(4):
                        ci = u * 4 + j
                        nc.tensor.matmul(
                            cz_ps,
                            kt[:, j, :],
                            vt[:, j, 0 : HID + 1],
                            start=(ci == 0),
                            stop=(ci == NCHUNK - 1),
                        )
                    # B1: S broadcast to 128 rows via block mask, then 1/S
                    sb_ps = apsum.tile([128, TILE_N], F32, tag="sb", bufs=1)
                    nc.tensor.matmul(
                        sb_ps, bmask_sb, eq_all[:, u, :], start=True, stop=True
                    )
                    rs = kpool.tile([128, TILE_N], F32, tag="rs")
                    nc.vector.reciprocal_approx_fast(rs, sb_ps)
                    rss[u] = rs
                if t >= 2:
                    u2 = t - 2
                    nc.vector.tensor_mul(
                        eqn_all[:, u2, :], eq_all[:, u2, :], rss.pop(u2)
                    )

        # ------- barrier: M = blockdiag(C/Z)^T @ WoT ------------------------
        with tc.tile_pool(name="barp", bufs=1, space="PSUM") as barpsum:
            rz = cpool.tile([128, 1], F32)
            nc.vector.reciprocal(rz, cz_ps[:, HID : HID + 1])
            bdf = cpool.tile([128, HID], F32)
            nc.vector.tensor_scalar_mul(bdf, cz_ps[:, 0:HID], rz)
            nc.vector.tensor_mul(bd_sb, bdf, bmask_sb)
            bdt_ps = barpsum.tile([128, HID], BF16)
            nc.tensor.transpose(bdt_ps, bd_sb, ident_sb)
            nc.vector.tensor_copy(bdt_sb, bdt_ps)
            m_ps = barpsum.tile([128, DIM], F32)
            nc.tensor.matmul(m_ps, bdt_sb, wot_sb, start=True, stop=True)
            nc.vector.tensor_copy(m_sb, m_ps)
        czctx.close()

        # ------- Phase B2: y = M^T @ eqn, evacuation split ScalarE/DVE ------
        with (
            tc.tile_pool(name="bsb", bufs=3) as bpool,
            tc.tile_pool(name="bps", bufs=2, space="PSUM") as bpsum,
        ):
            for t in range(NT):
                y_ps = bpsum.tile([128, 2, TILE_N], F32, tag="y", bufs=3)
                for oc in range(2):
                    nc.tensor.matmul(
                        y_ps[:, oc, :],
                        m_sb[:, oc * HID : (oc + 1) * HID],
                        eqn_all[:, t, :],
                        start=True,
                        stop=True,
                    )
                yt = bpool.tile([128, 2 * TILE_N], BF16, tag="yt")
                if bias_zero:
                    nc.scalar.copy(yt[:, 0:TILE_N], y_ps[:, 0, :])
                    nc.vector.tensor_copy(yt[:, TILE_N:], y_ps[:, 1, :])
                else:
                    nc.scalar.add(yt[:, 0:TILE_N], y_ps[:, 0, :], bo_sb[:, 0:1])
                    nc.vector.tensor_scalar_add(
                        yt[:, TILE_N:], y_ps[:, 1, :], bo_sb[:, 1:2]
                    )
                nc.sync.dma_start(y[t], yt)

    nc.compile()
    return nc


_GRAPH_CACHE: dict = {}


def _prep_inputs(x, context, Wq, Wkv, Wo, bo):
    bf16 = ml_dtypes.bfloat16
    x = np.asarray(x, dtype=np.float32)
    context = np.asarray(context, dtype=np.float32)
    Wq = np.asarray(Wq, dtype=np.float32)
    Wkv = np.asarray(Wkv, dtype=np.float32)
    Wo = np.asarray(Wo, dtype=np.float32)
    bo = np.asarray(bo, dtype=np.float32)

    # [128, 2, HID]: wqt[p, cc, m] = Wq[m, cc*128+p]
    wqt = np.ascontiguousarray(Wq.T.reshape(2, 128, HID).transpose(1, 0, 2)).astype(bf16)
    # [128, 4, 256]: wkvt[p, cc, o] = Wkv[o, cc*128+p]
    wkvt = np.ascontiguousarray(Wkv.T.reshape(4, 128, 2 * HID).transpose(1, 0, 2)).astype(bf16)
    # [HID, DIM]: wot[e, o] = Wo[o, e] * SCALE
    wot = np.ascontiguousarray((Wo * SCALE).T).astype(bf16)
    bob = np.ascontiguousarray(bo.reshape(2, 128).T).astype(np.float32)

    bmask = (
        (np.arange(HID)[:, None] // DIM_HEAD) == (np.arange(HID)[None, :] // DIM_HEAD)
    ).astype(bf16)
    ident = np.eye(HID, dtype=bf16)

    in_maps = []
    for b in range(B):
        # ctxt[t, p, cc*512+j] = context[b, t*512+j, cc*128+p]
        ctx_t = np.ascontiguousarray(
            context[b].reshape(NT, TILE_N, 4, 128).transpose(0, 3, 2, 1).reshape(NT, 128, 4 * TILE_N)
        ).astype(ml_dtypes.float8_e3m4)
        # xst[t, p, cc*512+j] = x[b, cc*128+p, t*512+j]
        xs_t = np.ascontiguousarray(
            x[b].reshape(2, 128, NT, TILE_N).transpose(2, 1, 0, 3).reshape(NT, 128, 2 * TILE_N)
        ).astype(bf16)
        in_maps.append(
            {
                "ctxt": ctx_t,
                "xst": xs_t,
                "wqt": wqt,
                "wkvt": wkvt,
                "wot": wot,
                "bob": bob,
                "bmask": bmask,
                "ident": ident,
            }
        )
    return in_maps


def run(inputs: dict, trace: bool = False):
    bias_zero = bool(np.all(np.asarray(inputs["bo"]) == 0))
    key = ("nc", bias_zero)
    if key not in _GRAPH_CACHE:
        _GRAPH_CACHE[key] = build_graph(bias_zero)
    nc = _GRAPH_CACHE[key]
    in_maps = _prep_inputs(**inputs)
    res = run_bass_kernel_spmd(nc, in_maps, core_ids=list(range(B)), trace=trace)
    out = np.stack(
        [
            np.asarray(res.results[b]["y"], dtype=np.float32)
            .reshape(NT, 128, 2, TILE_N)
            .transpose(2, 1, 0, 3)
            .reshape(DIM, N)
            for b in range(B)
        ]
    )
    return out, res


def kernel(**inputs) -> np.ndarray:
    out, _ = run(inputs, trace=False)
    return out


# revision 21
# speedup vs baseline: 1.3579x; 1.1490x over previous
"""Trainium2 Bass kernel for cross-attention (efficient/linear attention variant).

Computation per batch b (fully batch-independent -> data parallel over 8 cores):
    q  = Wq @ x[b]                         # (128, N)
    kv = Wkv @ context[b].T                # (256, NCTX)
    k, v = kv[:128], kv[128:]
    q = softmax_d(q) * d**-0.5             # softmax over feature dim within head
    k = softmax_n(k)                       # softmax over sequence dim
    ctx[h] = k_h @ v_h.T                   # (32, 32) per head
    out[h] = ctx[h].T @ q_h                # (32, N)
    y = Wo @ out + bo

Strategy (v4):
  - One batch per NeuronCore (8 cores), no collectives.
  - Host pre-transposes/tiles everything so all device DMAs are fully
    contiguous per partition; streams in bf16.
  - Merged phase A+B1 loop over 32 tiles: kvT = ctxT_chunk.T @ WkvT (n on
    partitions), exp(k) on ScalarE (one batched ACTIVATE per tile), one
    accumulating matmul per 128-chunk computes C[(d),(e)] = sum_n exp(k) v
    AND Z[d] = sum_n exp(k) via a ones column on v.  Concurrently (B1) the
    q-projection, exp(q), S = head-sums of exp(q) (computed PRE-BROADCAST
    to all 128 partitions via a block-mask matmul), and 1/S via
    reciprocal_approx_fast on DVE run on the same tiles using spare
    Scalar/DVE/PE capacity under the kv-matmul stream.  (A (4,n)-shaped
    reciprocal would serialize ~128us on 4 partitions; the broadcast-first
    form keeps all 128 DVE lanes busy.)
  - Barrier: BD = blockdiag(C/Z); M = BD^T @ (Wo^T * scale) folded once on
    device (PE transpose + 1 matmul), so phase B2 has no ctx matmul.
  - Phase B2 (2-stage pipeline): eqn = exp(q) * (1/S) on DVE,
    y = M^T @ eqn, one fused PSUM->SBUF evacuation on ScalarE (bias folded
    in; plain copy when bo == 0), DMA out.
    No Ln/Reciprocal activations anywhere -> zero ACT table swaps.
"""

import sys
from contextlib import ExitStack

import numpy as np

if "/opt/trn_rl_repo" not in sys.path:
    sys.path.insert(0, "/opt/trn_rl_repo")

import ml_dtypes

import concourse.bass as bass
from concourse import bacc
import concourse.mybir as mybir
import concourse.tile as tile
from concourse.bass_utils import run_bass_kernel_spmd

HEADS = 4
DIM_HEAD = 32
SCALE = DIM_HEAD**-0.5
B = 8
DIM = 256
N = 16384
NCTX = 16384
CDIM = 512
HID = HEADS * DIM_HEAD  # 128

BF16 = mybir.dt.bfloat16
F32 = mybir.dt.float32
FP8 = mybir.dt.float8e3
EXP = mybir.ActivationFunctionType.Exp

TILE_N = 512
NT = N // TILE_N  # 32 x-tiles (== context tiles)
NCHUNK = NCTX // 128  # 128 chunks in the C/Z accumulation


def build_graph(bias_zero: bool) -> bass.Bass:
    nc = bacc.Bacc()

    ctxt = nc.dram_tensor("ctxt", [NT, 128, 4 * TILE_N], FP8, kind="ExternalInput")
    xst = nc.dram_tensor("xst", [NT, 128, 2 * TILE_N], BF16, kind="ExternalInput")
    wqt = nc.dram_tensor("wqt", [128, 2, HID], BF16, kind="ExternalInput")
    wkvt = nc.dram_tensor("wkvt", [128, 4, 2 * HID], BF16, kind="ExternalInput")
    wot = nc.dram_tensor("wot", [HID, DIM], BF16, kind="ExternalInput")
    bob = nc.dram_tensor("bob", [128, 2], F32, kind="ExternalInput")
    bmask = nc.dram_tensor("bmask", [HID, HID], BF16, kind="ExternalInput")
    ident = nc.dram_tensor("ident", [HID, HID], BF16, kind="ExternalInput")
    y = nc.dram_tensor("y", [NT, 128, 2 * TILE_N], BF16, kind="ExternalOutput")

    with tile.TileContext(nc) as tc, ExitStack() as ctx:
        cpool = ctx.enter_context(tc.tile_pool(name="consts", bufs=1))

        # wkvt first: needed by the very first matmul.
        wkvt_sb = cpool.tile([128, 4, 2 * HID], BF16)
        nc.sync.dma_start(wkvt_sb, wkvt[:, :, :])

        # persistent intermediates
        eq_all = cpool.tile([128, NT, TILE_N], BF16)  # exp(q), 32KB/part
        eqn_all = cpool.tile([128, NT, TILE_N], BF16)  # exp(q)/S, 32KB/part
        m_sb = cpool.tile([HID, DIM], BF16)  # folded BD^T @ WoT
        bd_sb = cpool.tile([HID, HID], BF16)
        bdt_sb = cpool.tile([HID, HID], BF16)

        # manual vt rotation: ones column at [:, :, HID] preset once
        vt_bufs = [cpool.tile([128, 4, 136], BF16, name=f"vtb{i}") for i in range(3)]
        for vtb in vt_bufs:
            nc.gpsimd.memset(vtb[:, :, HID : HID + 1], 1.0)

        czctx = ExitStack()
        czpool = czctx.enter_context(tc.tile_pool(name="czp", bufs=1, space="PSUM"))
        cz_ps = czpool.tile([128, HID + 1], F32)

        with (
            tc.tile_pool(name="actx", bufs=4) as apool,
            tc.tile_pool(name="axs", bufs=6) as xpool,
            tc.tile_pool(name="akv", bufs=3) as kpool,
            tc.tile_pool(name="aps", bufs=2, space="PSUM") as apsum,
        ):
            # context tile 0 (split in two for earlier compute start) + x0
            ct_tiles: dict = {}
            xs_tiles: dict = {}
            ct0 = apool.tile([128, 4 * TILE_N], FP8, tag="ct")
            ct0_r = ct0.rearrange("p (c n) -> p c n", c=4)
            src0 = ctxt[0].rearrange("p (c n) -> p c n", c=4)
            # first context tile issued on the scalar HWDGE ring, in parallel
            # with wkvt on the sync ring, so the first matmul starts earlier
            nc.scalar.dma_start(ct0_r[:, :, 0:256], src0[:, :, 0:256])
            nc.scalar.dma_start(ct0_r[:, :, 256:TILE_N], src0[:, :, 256:TILE_N])
            xs0 = xpool.tile([128, 2 * TILE_N], BF16, tag="xt")
            nc.sync.dma_start(xs0, xst[0])
            ct_tiles[0] = ct0
            xs_tiles[0] = xs0

            wqt_sb = cpool.tile([128, 2, HID], BF16)
            nc.scalar.dma_start(wqt_sb, wqt[:, :, :])
            wot_sb = cpool.tile([HID, DIM], BF16)
            nc.sync.dma_start(wot_sb, wot[:, :])
            bo_sb = cpool.tile([128, 2], F32)
            nc.sync.dma_start(bo_sb, bob[:, :])
            bmask_sb = cpool.tile([HID, HID], BF16)
            nc.scalar.dma_start(bmask_sb, bmask[:, :])
            ident_sb = cpool.tile([HID, HID], BF16)
            nc.sync.dma_start(ident_sb, ident[:, :])

            # ------- merged Phase A + B1 (C/S work staggered one tile) ------
            kts: dict = {}
            rss: dict = {}
            for t in range(NT + 2):
                if t < NT:
                    if t not in ct_tiles:
                        ct = apool.tile([128, 4 * TILE_N], FP8, tag="ct")
                        nc.sync.dma_start(ct, ctxt[t])
                        xs = xpool.tile([128, 2 * TILE_N], BF16, tag="xt")
                        nc.sync.dma_start(xs, xst[t])
                        ct_tiles[t] = ct
                        xs_tiles[t] = xs
                    ct = ct_tiles.pop(t)
                    xs = xs_tiles.pop(t)
                    # kv projection: 4 chunks of 128 n, contraction over 4 cc
                    kvt_ps = apsum.tile([128, 4, 2 * HID], F32, tag="kvt")
                    for j in range(4):
                        for cc in range(4):
                            nc.tensor.matmul(
                                kvt_ps[:, j, :],
                                ct[:, cc * TILE_N + j * 128 : cc * TILE_N + (j + 1) * 128],
                                wkvt_sb[:, cc, :],
                                start=(cc == 0),
                                stop=(cc == 3),
                            )
                    kt = kpool.tile([128, 4, HID], BF16, tag="kt")
                    nc.scalar.activation(kt, kvt_ps[:, :, 0:HID], EXP)
                    vt = vt_bufs[t % 3]
                    nc.vector.tensor_copy(vt[:, :, 0:HID], kvt_ps[:, :, HID : 2 * HID])
                    kts[t] = kt
                    # B1: q projection + exp(q)
                    q_ps = apsum.tile([128, TILE_N], F32, tag="q")
                    for cc in range(2):
                        nc.tensor.matmul(
                            q_ps,
                            wqt_sb[:, cc, :],
                            xs[:, cc * TILE_N : (cc + 1) * TILE_N],
                            start=(cc == 0),
                            stop=(cc == 1),
                        )
                    nc.scalar.activation(eq_all[:, t, :], q_ps, EXP)
                if 1 <= t <= NT:
                    u = t - 1
                    kt = kts.pop(u)
                    vt = vt_bufs[u % 3]
                    for j in range(4):
                        ci = u * 4 + j
                        nc.tensor.matmul(
                            cz_ps,
                            kt[:, j, :],
                            vt[:, j, 0 : HID + 1],
                            start=(ci == 0),
                            stop=(ci == NCHUNK - 1),
                        )
                    # B1: S broadcast to 128 rows via block mask, then 1/S
                    sb_ps = apsum.tile([128, TILE_N], F32, tag="sb", bufs=1)
                    nc.tensor.matmul(
                        sb_ps, bmask_sb, eq_all[:, u, :], start=True, stop=True
                    )
                    rs = kpool.tile([128, TILE_N], F32, tag="rs")
                    nc.vector.reciprocal_approx_fast(rs, sb_ps)
                    rss[u] = rs
                if t >= 2:
                    u2 = t - 2
                    nc.vector.tensor_mul(
                        eqn_all[:, u2, :], eq_all[:, u2, :], rss.pop(u2)
                    )

        # ------- barrier: M = blockdiag(C/Z)^T @ WoT ------------------------
        with tc.tile_pool(name="barp", bufs=1, space="PSUM") as barpsum:
            rz = cpool.tile([128, 1], F32)
            nc.vector.reciprocal(rz, cz_ps[:, HID : HID + 1])
            bdf = cpool.tile([128, HID], F32)
            nc.vector.tensor_scalar_mul(bdf, cz_ps[:, 0:HID], rz)
            nc.vector.tensor_mul(bd_sb, bdf, bmask_sb)
            bdt_ps = barpsum.tile([128, HID], BF16)
            nc.tensor.transpose(bdt_ps, bd_sb, ident_sb)
            nc.vector.tensor_copy(bdt_sb, bdt_ps)
            m_ps = barpsum.tile([128, DIM], F32)
            nc.tensor.matmul(m_ps, bdt_sb, wot_sb, start=True, stop=True)
            nc.vector.tensor_copy(m_sb, m_ps)
        czctx.close()

        # ------- Phase B2: y = M^T @ eqn, evacuation split ScalarE/DVE ------
        with (
            tc.tile_pool(name="bsb", bufs=3) as bpool,
            tc.tile_pool(name="bps", bufs=2, space="PSUM") as bpsum,
        ):
            for t in range(NT):
                y_ps = bpsum.tile([128, 2, TILE_N], F32, tag="y", bufs=3)
                for oc in range(2):
                    nc.tensor.matmul(
                        y_ps[:, oc, :],
                        m_sb[:, oc * HID : (oc + 1) * HID],
                        eqn_all[:, t, :],
                        start=True,
                        stop=True,
                    )
                yt = bpool.tile([128, 2 * TILE_N], BF16, tag="yt", bufs=8)
                if bias_zero:
                    nc.scalar.copy(yt[:, 0:TILE_N], y_ps[:, 0, :])
                    nc.vector.tensor_copy(yt[:, TILE_N:], y_ps[:, 1, :])
                else:
                    nc.scalar.add(yt[:, 0:TILE_N], y_ps[:, 0, :], bo_sb[:, 0:1])
                    nc.vector.tensor_scalar_add(
                        yt[:, TILE_N:], y_ps[:, 1, :], bo_sb[:, 1:2]
                    )
                nc.sync.dma_start(y[t], yt)

    nc.compile()
    return nc


_GRAPH_CACHE: dict = {}


def _prep_inputs(x, context, Wq, Wkv, Wo, bo):
    bf16 = ml_dtypes.bfloat16
    x = np.asarray(x, dtype=np.float32)
    context = np.asarray(context, dtype=np.float32)
    Wq = np.asarray(Wq, dtype=np.float32)
    Wkv = np.asarray(Wkv, dtype=np.float32)
    Wo = np.asarray(Wo, dtype=np.float32)
    bo = np.asarray(bo, dtype=np.float32)

    # [128, 2, HID]: wqt[p, cc, m] = Wq[m, cc*128+p]
    wqt = np.ascontiguousarray(Wq.T.reshape(2, 128, HID).transpose(1, 0, 2)).astype(bf16)
    # [128, 4, 256]: wkvt[p, cc, o] = Wkv[o, cc*128+p]
    wkvt = np.ascontiguousarray(Wkv.T.reshape(4, 128, 2 * HID).transpose(1, 0, 2)).astype(bf16)
    # [HID, DIM]: wot[e, o] = Wo[o, e] * SCALE
    wot = np.ascontiguousarray((Wo * SCALE).T).astype(bf16)
    bob = np.ascontiguousarray(bo.reshape(2, 128).T).astype(np.float32)

    bmask = (
        (np.arange(HID)[:, None] // DIM_HEAD) == (np.arange(HID)[None, :] // DIM_HEAD)
    ).astype(bf16)
    ident = np.eye(HID, dtype=bf16)

    in_maps = []
    for b in range(B):
        # ctxt[t, p, cc*512+j] = context[b, t*512+j, cc*128+p]
        ctx_t = np.ascontiguousarray(
            context[b].reshape(NT, TILE_N, 4, 128).transpose(0, 3, 2, 1).reshape(NT, 128, 4 * TILE_N)
        ).astype(ml_dtypes.float8_e3m4)
        # xst[t, p, cc*512+j] = x[b, cc*128+p, t*512+j]
        xs_t = np.ascontiguousarray(
            x[b].reshape(2, 128, NT, TILE_N).transpose(2, 1, 0, 3).reshape(NT, 128, 2 * TILE_N)
        ).astype(bf16)
        in_maps.append(
            {
                "ctxt": ctx_t,
                "xst": xs_t,
                "wqt": wqt,
                "wkvt": wkvt,
                "wot": wot,
                "bob": bob,
                "bmask": bmask,
                "ident": ident,
            }
        )
    return in_maps


def run(inputs: dict, trace: bool = False):
    bias_zero = bool(np.all(np.asarray(inputs["bo"]) == 0))
    key = ("nc", bias_zero)
    if key not in _GRAPH_CACHE:
        _GRAPH_CACHE[key] = build_graph(bias_zero)
    nc = _GRAPH_CACHE[key]
    in_maps = _prep_inputs(**inputs)
    res = run_bass_kernel_spmd(nc, in_maps, core_ids=list(range(B)), trace=trace)
    out = np.stack(
        [
            np.asarray(res.results[b]["y"], dtype=np.float32)
            .reshape(NT, 128, 2, TILE_N)
            .transpose(2, 1, 0, 3)
            .reshape(DIM, N)
            for b in range(B)
        ]
    )
    return out, res


def kernel(**inputs) -> np.ndarray:
    out, _ = run(inputs, trace=False)
    return out
